# revision 46
# baseline (speedup 1.0000x reference)
"""Expert-parallel MoE layer for Trainium2 (8 NeuronCores).

Host side (numpy): router logits, exact top-2 dispatch, p0 weights, and the
scatter-add combine. Device side (Bass/Tile, SPMD over 8 cores): the dense FFN
y = gelu(x @ W1[e] + b1[e]) @ W2[e] in bf16 (fp32 PSUM accumulation).

Load balancing: one expert per core would bound every core by the largest
expert's token count (2182 for the seed-0 routing). Instead each core runs
K segments (K weight sets, K=2 or 3 chosen by a small exact solver). Each
expert's tokens split into whole slots of the (8 cores x K segments) grid;
the solver picks per-segment column caps (multiples of 16 for SBUF
alignment) minimizing the per-core total, e.g. (736, 704, 624) = 2064
slots/core for seed-0 — a 5.4% PE-floor reduction over 2182.

Per-core pipeline: F (4096) is processed in NQ=4 quarters with per-segment
W1/W2 quarter slices streamed through SBUF; x stays resident in bf16. GEMM2
is computed transposed (stationary = W2 chunk, moving = hT) so PE cost scales
with the exact token count, and y^T accumulates across quarters in SBUF (no
DRAM read-modify-write). GEMM1 runs one token-group ahead of GEMM2
(software pipeline) to hide activation latency and startup DMA. All inputs
are host-packed in exact SBUF layout so DMA rows are long contiguous runs
(DMA here is per-row-overhead-bound). bf16 (not fp16) keeps the PE further
from its power-throttle ceiling.
"""

import numpy as np

B, S, H, E, F = 4, 2048, 1024, 8, 4096
T = B * S
P = 128
NQ = 4              # F quarters (outer loop)
FQ = F // NQ
KH = H // P         # k-chunks over H (GEMM1 contraction)
KFQ = FQ // P       # k-chunks over one F quarter (GEMM2 contraction)
HC = H // P         # output H chunks (GEMM2 transposed: psum partition = h')
TT = 512            # token group (matmul moving free dim)

_cache = {}
_solve_cache = {}


def _spill_waits(nc, mybir, max_waits=1):
    """walrus CoreV2/V3 codegen rejects instructions with >1 semaphore wait
    ("Too many sync wait commands") — notably self-loading fp32/fp32r matmuls
    and DMACopy. Move excess waits onto same-engine no-ops inserted right
    before the instruction (sequencers run in order, so this is equivalent)."""
    for fn in nc.m.functions:
        for blk in fn.blocks:
            out = []
            changed = False
            for inst in blk.instructions:
                si = getattr(inst, "sync_info", None)
                if si is not None and len(si.on_wait) > max_waits:
                    spill = si.on_wait[: len(si.on_wait) - max_waits]
                    keep = si.on_wait[len(si.on_wait) - max_waits:]
                    for w in spill:
                        nop = mybir.InstNoOp(
                            name=nc.get_next_instruction_name(),
                            engine=inst.engine,
                            ins=[],
                            outs=[],
                        )
                        nop.sync_info = mybir.SyncInfo(on_wait=[w], on_update=[])
                        out.append(nop)
                    inst.sync_info = mybir.SyncInfo(on_wait=keep, on_update=si.on_update)
                    changed = True
                out.append(inst)
            if changed:
                blk.instructions = out


def _groups(cap, first_small):
    """Token groups within one segment: optionally a small first group (so
    the startup DMA for it is tiny and the PE starts early), 512s after, and
    no tiny remainder group (tiny groups are LDWEIGHTS-bound). All sizes are
    multiples of 16 elements — odd sizes make the strided x slices lose 16B
    SBUF alignment and the matmul stream slows ~2x."""
    assert cap % 16 == 0
    sizes = []
    o = 0
    while o < cap:
        tt = min(256 if (first_small and not sizes) else TT, cap - o)
        sizes.append(tt)
        o += tt
    if len(sizes) >= 2 and sizes[-1] < 280:
        pair = sizes[-2] + sizes[-1]
        half = -(-pair // 32) * 16
        sizes[-2] = half
        sizes[-1] = pair - half
    groups = []
    o = 0
    for tt in sizes:
        groups.append((o, tt))
        o += tt
    return groups


def _solve_caps(counts, ncores):
    """Pick per-segment column caps (multiples of 16) and a slot assignment
    minimizing total cap. Tries 2 and 3 segments, exhaustively over c16 cap
    tuples in ascending total order; feasibility via DP."""

    def c16(v):
        return -(-v // 16) * 16

    total = sum(counts)
    lb = c16(-(-total // ncores))
    hi_single = c16(max(counts))
    for cap in range(lb, hi_single + 1, 16):
        for K in (2, 3):
            tuples = []
            if K == 2:
                for ca in range(c16(cap // 2), cap - 255, 16):
                    tuples.append((ca, cap - ca))
            else:
                for ca in range(c16(cap // 3), cap - 511, 16):
                    for cb in range(c16((cap - ca) // 2), min(ca, cap - ca - 255) + 1, 16):
                        tuples.append((ca, cb, cap - ca - cb))
            for caps in tuples:
                if min(caps) < 256:
                    continue
                picks = _dp_solve(counts, caps, ncores)
                if picks is not None:
                    return caps, picks
    # fallback: one expert per core
    return (hi_single,), [tuple([1]) for _ in counts]


def _dp_solve(counts, caps, ncores):
    """Forward DP with parent pointers; returns per-expert slot-count
    vectors (one entry per column) or None."""
    from itertools import product

    K = len(caps)
    options = []
    for cnt in counts:
        opts = []
        for d in product(range(min(ncores, 3) + 1), repeat=K):
            sd = sum(d)
            if sd == 0 or sd > 4:
                continue
            if sum(dj * c for dj, c in zip(d, caps)) < cnt:
                continue
            opts.append(d)
        if not opts:
            return None
        options.append(opts)

    layers = [{tuple([0] * K): None}]
    for i in range(len(counts)):
        nxt = {}
        for st in layers[-1]:
            for d in options[i]:
                ns = tuple(st[j] + d[j] for j in range(K))
                if any(v > ncores for v in ns):
                    continue
                if ns not in nxt:
                    nxt[ns] = (st, d)
        if not nxt:
            return None
        layers.append(nxt)
    goal = tuple([ncores] * K)
    if goal not in layers[-1]:
        return None
    picks = [None] * len(counts)
    st = goal
    for i in range(len(counts) - 1, -1, -1):
        prev, d = layers[i + 1][st]
        picks[i] = d
        st = prev
    return picks


def _assign_slots(counts, caps, picks, ncores):
    """Concrete (expert, start, length) slots per column, ncores each."""
    cols = [[] for _ in caps]
    for i, cnt in enumerate(counts):
        off = 0
        rem = cnt
        slots = [(j, caps[j]) for j in range(len(caps)) for _ in range(picks[i][j])]
        # fill the biggest columns first so no slot overflows
        slots.sort(key=lambda s: -s[1])
        for j, cj in slots:
            ln = min(cj, rem)
            cols[j].append((i, off, ln))
            off += ln
            rem -= ln
        assert rem == 0
    for j, col in enumerate(cols):
        assert len(col) == ncores, (j, len(col))
    return cols


def _build(*caps):
    import concourse.bass as bass
    import concourse.mybir as mybir
    from concourse import tile

    F32 = mybir.dt.float32
    SDT = mybir.dt.bfloat16
    GELU = mybir.ActivationFunctionType.Gelu_apprx_tanh
    ADD = mybir.AluOpType.add

    nseg = len(caps)
    cap = sum(caps)
    seg_groups = [_groups(c, seg == 0) for seg, c in enumerate(caps)]
    # flat work list: (seg, gi, global t0, tt, x offset in packed layout)
    work = []
    xo = 0
    base = 0
    for seg, sg in enumerate(seg_groups):
        for gi, (t0, tt) in enumerate(sg):
            work.append((seg, gi, base + t0, tt, xo))
            xo += KH * tt
        base += caps[seg]

    QW1 = KFQ * KH * P       # one quarter of packed W1, per partition
    QW2 = KFQ * H
    nc = bass.Bass()
    xt = nc.declare_dram_parameter("xt", [P, KH * cap], SDT, isOutput=False)
    w1 = nc.declare_dram_parameter("w1", [P, nseg * NQ * QW1], SDT, isOutput=False)
    w2 = nc.declare_dram_parameter("w2", [P, nseg * NQ * QW2], SDT, isOutput=False)
    b1s = nc.declare_dram_parameter("b1s", [P, nseg * F // P], F32, isOutput=False)
    yt = nc.declare_dram_parameter("yt", [H, cap], F32, isOutput=True)

    with tile.TileContext(nc) as tc:
        with (
            tc.tile_pool(name="w1p", bufs=2) as w1p,
            tc.tile_pool(name="w2p", bufs=2) as w2p,
            tc.tile_pool(name="xp", bufs=1) as xp,
            tc.tile_pool(name="hp", bufs=1) as hp,
            tc.tile_pool(name="yp", bufs=1) as yp,
            tc.tile_pool(name="cst", bufs=1) as cst,
            tc.tile_pool(name="ps1", bufs=4, space="PSUM") as ps1,
            tc.tile_pool(name="ps2", bufs=4, space="PSUM") as ps2,
        ):
            def load_w1(seg, q):
                w1q = w1p.tile([P, KFQ, KH, P], SDT, tag="w1q")
                o = (seg * NQ + q) * QW1
                nc.sync.dma_start(w1q[:], w1[:, o:o + QW1])
                return w1q

            def load_w2(seg, q, split=False):
                # DMA issues ride gpsimd/sync so the Scalar engine (which
                # runs the critical-path activations) never pays descriptors
                w2q = w2p.tile([P, KFQ, H], SDT, tag="w2q")
                o = (seg * NQ + q) * QW2
                src = w2[:, o:o + QW2]
                if split:
                    nc.gpsimd.dma_start(w2q[:, : KFQ // 2, :], src[:, : QW2 // 2])
                    nc.sync.dma_start(w2q[:, KFQ // 2:, :], src[:, QW2 // 2:])
                else:
                    nc.gpsimd.dma_start(w2q[:], src)
                return w2q

            # prologue. DMA throughput is row-overhead-bound, so feed order
            # matters most: x(first group) on scalar, then W1(segA, q0) as 8
            # fs-blocks alternating sync/gpsimd (each immediately consumed by
            # one GEMM1 chain), W2(segA, q0) halves behind them, remaining x
            # groups on scalar.
            b1t = cst.tile([P, nseg * F // P], F32)
            x_all = xp.tile([P, KH * cap], SDT)
            nc.scalar.dma_start(x_all[:, : KH * work[0][3]],
                                xt[:, : KH * work[0][3]])
            nc.scalar.dma_start(b1t[:], b1s[:])
            w1q0 = w1p.tile([P, KFQ, KH, P], SDT, tag="w1q")
            BL = KH * P
            for fs in range(KFQ):
                wq = nc.sync if fs % 2 == 0 else nc.gpsimd
                wq.dma_start(w1q0[:, fs, :, :], w1[:, fs * BL:(fs + 1) * BL])
            w2q0 = load_w2(0, 0, split=True)
            for seg, gi, t0, tt, xo in work[1:]:
                nc.scalar.dma_start(x_all[:, xo:xo + KH * tt],
                                    xt[:, xo:xo + KH * tt])
            yT = yp.tile([P, HC, cap], F32)
            ytdram = yt.rearrange("(c p) t -> p c t", p=P)

            w1qs = {(0, 0): w1q0}
            w2qs = {(0, 0): w2q0}
            hqs = {}

            def gemm1(q, wi, idx):
                seg, gi, t0, tt, xo = work[wi]
                w1q = w1qs[(seg, q)]
                # hT[f, t] = sum_h W1[h, f] * xT[h, t], then gelu
                hq = hp.tile([P, KFQ, TT], SDT, tag=f"hq{idx % 2}")
                hqs[(q, wi)] = hq
                for fs in range(KFQ):
                    pt = ps1.tile([P, TT], F32, tag="pt1")
                    for k in range(KH):
                        nc.tensor.matmul(
                            pt[:, :tt],
                            w1q[:, fs, k, :],
                            x_all[:, xo + k * tt: xo + (k + 1) * tt],
                            start=(k == 0),
                            stop=(k == KH - 1),
                        )
                    c = (seg * NQ + q) * KFQ + fs
                    nc.scalar.activation(
                        hq[:, fs, :tt], pt[:, :tt], GELU, bias=b1t[:, c:c + 1]
                    )
                # weight prefetches: the next (seg, q) slot's 2MB loads issue
                # one-to-two groups before first use (hidden under ~20-40us
                # of compute), late enough not to fight the startup burst
                ng = len(seg_groups[seg])
                if seg + 1 < nseg:
                    nxt = (seg + 1, q)
                elif q + 1 < NQ:
                    nxt = (0, q + 1)
                else:
                    nxt = None
                if nxt is not None:
                    if gi == min(1, ng - 1) and (seg, gi) != (0, 0):
                        w1qs[(nxt[0], nxt[1])] = load_w1(*nxt)
                    if gi == min(2, ng - 1) and (seg, gi) != (0, 0):
                        w2qs[(nxt[0], nxt[1])] = load_w2(*nxt)

            def gemm2(q, wi):
                seg, gi, t0, tt, xo = work[wi]
                w2q = w2qs[(seg, q)]
                hq = hqs.pop((q, wi))
                # transposed: yT[h', t] += sum_f W2[f, h'] * hT[f, t]
                for hc in range(HC):
                    pt2 = ps2.tile([P, TT], F32, tag="pt2")
                    for k2 in range(KFQ):
                        nc.tensor.matmul(
                            pt2[:, :tt],
                            w2q[:, k2, hc * P:(hc + 1) * P],
                            hq[:, k2, :tt],
                            start=(k2 == 0),
                            stop=(k2 == KFQ - 1),
                        )
                    if q == 0:
                        nc.vector.tensor_copy(yT[:, hc, t0:t0 + tt], pt2[:, :tt])
                    else:
                        nc.vector.tensor_tensor(
                            yT[:, hc, t0:t0 + tt],
                            yT[:, hc, t0:t0 + tt],
                            pt2[:, :tt],
                            ADD,
                        )
                    if q == NQ - 1:
                        # y^T final for this (group, hc): stream it out as
                        # soon as its add lands so almost nothing drains
                        # after the last matmul; alternate queues so a single
                        # backlogged queue can't delay the end
                        yq = nc.sync if hc % 2 == 0 else nc.gpsimd
                        yq.dma_start(
                            ytdram[:, hc, t0:t0 + tt], yT[:, hc, t0:t0 + tt]
                        )

            # software pipeline: GEMM1 runs one work item ahead of GEMM2, so
            # GEMM2 never waits on its own group's activations and the PE
            # has a full group of GEMM1 work queued during startup DMA
            sched = [(q, wi) for q in range(NQ) for wi in range(len(work))]
            for idx, (q, wi) in enumerate(sched):
                gemm1(q, wi, idx)
                if idx >= 1:
                    gemm2(*sched[idx - 1])
            gemm2(*sched[-1])

    import concourse.mybir as mybir_mod

    _spill_waits(nc, mybir_mod)
    return nc


def _route(x2d, Wr, br):
    """Top-2 routing, bit-matching the reference's decisions.

    Softmax is monotonic, so top-2-of-probs == top-2-of-logits, and the
    normalized top-1 weight p0 = p1/(p1+p2) == sigmoid(l1-l2) exactly (the
    softmax denominator cancels). Ordering ties are broken by lower index,
    same as jax.lax.top_k."""
    logits = x2d @ np.asarray(Wr, np.float32) + np.asarray(br, np.float32)
    order = np.argsort(-logits, axis=-1, kind="stable")
    i1 = order[:, 0].astype(np.int64)
    i2 = order[:, 1].astype(np.int64)
    r = np.arange(logits.shape[0])
    l1 = logits[r, i1].astype(np.float64)
    l2 = logits[r, i2].astype(np.float64)
    p0 = 1.0 / (1.0 + np.exp(l2 - l1))
    return i1, i2, p0.astype(np.float32)


def _pack_w1(w1e):
    return (
        w1e.reshape(KH, P, NQ, KFQ, P)
        .transpose(1, 2, 3, 0, 4)
        .reshape(P, NQ * KFQ * KH * P)
    )


def _pack_w2(w2e):
    return (
        w2e.reshape(NQ, KFQ, P, H)
        .transpose(2, 0, 1, 3)
        .reshape(P, NQ * KFQ * H)
    )


def _plan(x, Wr, br, W1, b1, W2, b2):
    """Route on host; solve the 2-segment balance; build per-core inputs."""
    import ml_dtypes

    BF16 = ml_dtypes.bfloat16
    x2d = np.ascontiguousarray(np.asarray(x, np.float32).reshape(T, H))
    W1 = np.asarray(W1, np.float32)
    b1 = np.asarray(b1, np.float32)
    W2 = np.asarray(W2, np.float32)

    i1, i2, p0 = _route(x2d, Wr, br)
    idxs = [np.flatnonzero((i1 == e) | (i2 == e)) for e in range(E)]
    counts = [len(ix) for ix in idxs]

    ckey = tuple(counts)
    if ckey not in _solve_cache:
        _solve_cache[ckey] = _solve_caps(counts, E)
    caps, picks = _solve_cache[ckey]
    cols = _assign_slots(counts, caps, picks, E)
    cap = sum(caps)
    bases = [sum(caps[:j]) for j in range(len(caps))]
    seg_groups = [_groups(c, seg == 0) for seg, c in enumerate(caps)]

    w1pk = {e: _pack_w1(W1[e]).astype(BF16) for e in range(E)}
    w2pk = {e: _pack_w2(W2[e]).astype(BF16) for e in range(E)}

    xT = np.ascontiguousarray(x2d.T)  # [H, T]
    in_maps = []
    slot_info = []  # per core: [(expert, token_index_array, global t offset)]
    for c in range(E):
        xte = np.zeros((H, cap), np.float32)
        info = []
        segw1, segw2, segb1 = [], [], []
        for seg in range(len(caps)):
            e, s0, ln = cols[seg][c]
            base = bases[seg]
            ix = idxs[e][s0:s0 + ln]
            xte[:, base:base + ln] = xT[:, ix]
            info.append((e, ix, base))
            segw1.append(w1pk[e])
            segw2.append(w2pk[e])
            segb1.append(np.ascontiguousarray(b1[e].reshape(F // P, P).T))
        # pack x group-major in exact SBUF layout
        xr = xte.reshape(KH, P, cap)
        parts = []
        for seg, sg in enumerate(seg_groups):
            base = bases[seg]
            for t0, tt in sg:
                parts.append(
                    xr[:, :, base + t0:base + t0 + tt]
                    .transpose(1, 0, 2)
                    .reshape(P, KH * tt)
                )
        xpk = np.concatenate(parts, axis=1)
        in_maps.append(
            {
                "xt": np.ascontiguousarray(xpk).astype(BF16),
                "w1": np.ascontiguousarray(np.concatenate(segw1, axis=1)),
                "w2": np.ascontiguousarray(np.concatenate(segw2, axis=1)),
                "b1s": np.ascontiguousarray(np.concatenate(segb1, axis=1)),
            }
        )
        slot_info.append(info)
    return tuple(caps), in_maps, slot_info, p0


def kernel(x, Wr, br, W1, b1, W2, b2):
    from concourse.bass_utils import run_bass_kernel_spmd

    key, in_maps, slot_info, p0 = _plan(x, Wr, br, W1, b1, W2, b2)

    if key not in _cache:
        _cache[key] = _build(*key)
    nc = _cache[key]

    try:
        res = run_bass_kernel_spmd(nc, in_maps, list(range(E)))
    except Exception:
        import time as _time

        _time.sleep(10)
        res = run_bass_kernel_spmd(nc, in_maps, list(range(E)))

    b2 = np.asarray(b2, np.float32)
    out = np.zeros((T, H), np.float32)
    for c in range(E):
        yt = res.results[c]["yt"]  # [H, cap] fp32
        for e, ix, base in slot_info[c]:
            if len(ix):
                ye = yt[:, base:base + len(ix)].T
                out[ix] += p0[ix, None] * (ye + b2[e][None, :])
    return out.reshape(B, S, H)
